# revision 4
# baseline (speedup 1.0000x reference)
"""BoundaryLoss kernel for Trainium2 (8 NeuronCores, Bass/Tile).

loss = mean_{n, c in 1..3} sum_{h,w} softmax[n,c,h,w] * distance[n,c,h,w]

Strategy (pure data parallel, per sharding hint):
  - core k gets batches {2k, 2k+1}; host slices classes 1:4 (contiguous
    per batch) and reshapes each core's 12.6 MB of data to [128, 12288].
  - on-device: stream [128, FT] tile pairs, elementwise product on the
    vector engine, partition-reduction of each product tile via a
    ones-vector matmul accumulating into one PSUM bank on the tensor
    engine (tensor_tensor_reduce compiles but crashes the deployed ucode,
    so the reduction lives on the PE instead).
  - host: sum the 8 per-core partial sums, divide by 48.

Memory-bound: ~12.6 MB HBM reads per core -> ~35 us roofline at
~358 GB/s per-NC HBM bandwidth.
"""

import numpy as np

import concourse.bass as bass
import concourse.tile as tile
from concourse import bacc, mybir
from concourse.bass_utils import run_bass_kernel_spmd

N_CORES = 8
P = 128
N, C, H, W = 16, 4, 512, 512
CLS = C - 1                       # classes 1..3 (background excluded)
PER_CORE_N = N // N_CORES         # 2 batches per core
FREE = PER_CORE_N * CLS * H * W // P   # 12288 free elems per partition
FT = 2048                         # tile free-dim (1 MiB per [128, FT] f32 tile)
NT = FREE // FT

_nc_cache = None


def build_nc():
    global _nc_cache
    if _nc_cache is not None:
        return _nc_cache

    nc = bacc.Bacc(None, target_bir_lowering=False)
    f32 = mybir.dt.float32
    sm = nc.dram_tensor("sm", [P, FREE], f32, kind="ExternalInput")
    dm = nc.dram_tensor("dm", [P, FREE], f32, kind="ExternalInput")
    out = nc.dram_tensor("out", [1, 1], f32, kind="ExternalOutput")

    MM = 512  # one PSUM bank / max moving free dim per matmul

    with tile.TileContext(nc) as tc:
        with (
            tc.tile_pool(name="a", bufs=3) as pa,
            tc.tile_pool(name="b", bufs=3) as pb,
            tc.tile_pool(name="prod", bufs=2) as pp,
            tc.tile_pool(name="misc", bufs=1) as pm,
            tc.tile_pool(name="ps", bufs=1, space="PSUM") as pps,
        ):
            ones = pm.tile([P, 1], f32)
            nc.gpsimd.memset(ones[:], 1.0)
            psacc = pps.tile([1, MM], f32)

            for t in range(NT):
                ta = pa.tile([P, FT], f32)
                tb = pb.tile([P, FT], f32)
                # alternate the two HWDGE rings (SP / ACT)
                eng_a = nc.sync if (t % 2 == 0) else nc.scalar
                eng_b = nc.scalar if (t % 2 == 0) else nc.sync
                eng_a.dma_start(ta[:], sm[:, bass.ts(t, FT)])
                eng_b.dma_start(tb[:], dm[:, bass.ts(t, FT)])

                prod = pp.tile([P, FT], f32)
                nc.vector.tensor_mul(prod[:], ta[:], tb[:])
                # psacc[0, j] += sum_p prod[p, j] for each 512-col chunk,
                # accumulated across all tiles in one PSUM bank.
                for j in range(FT // MM):
                    nc.tensor.matmul(
                        psacc[:],
                        ones[:],
                        prod[:, bass.ts(j, MM)],
                        start=(t == 0 and j == 0),
                        stop=(t == NT - 1 and j == FT // MM - 1),
                    )

            res = pm.tile([1, 1], f32)
            nc.vector.reduce_sum(res[:], psacc[:], axis=mybir.AxisListType.X)
            nc.sync.dma_start(out[:], res[:])

    nc.compile()
    _nc_cache = nc
    return nc


def make_in_maps(softmax_output, distance_maps):
    sm = np.ascontiguousarray(softmax_output[:, 1:, :, :]).reshape(N, CLS * H * W)
    dm = np.ascontiguousarray(distance_maps[:, 1:, :, :]).reshape(N, CLS * H * W)
    in_maps = []
    for k in range(N_CORES):
        rows = slice(k * PER_CORE_N, (k + 1) * PER_CORE_N)
        in_maps.append(
            {
                "sm": sm[rows].reshape(P, FREE),
                "dm": dm[rows].reshape(P, FREE),
            }
        )
    return in_maps


def run(softmax_output, distance_maps, **spmd_kwargs):
    """Returns (loss ndarray, BassKernelResults)."""
    nc = build_nc()
    in_maps = make_in_maps(softmax_output, distance_maps)
    r = run_bass_kernel_spmd(nc, in_maps, core_ids=list(range(N_CORES)), **spmd_kwargs)
    total = sum(float(res["out"][0, 0]) for res in r.results)
    loss = np.float32(total / (N * CLS))
    return np.asarray(loss, dtype=np.float32), r


def kernel(softmax_output, target, distance_maps):
    softmax_output = np.asarray(softmax_output, dtype=np.float32)
    distance_maps = np.asarray(distance_maps, dtype=np.float32)
    loss, _ = run(softmax_output, distance_maps)
    return loss
